# revision 5
# baseline (speedup 1.0000x reference)
"""Trainium2 Bass kernel for LoraLinear:
    out = x @ W^T + 2.0 * (x @ A^T) @ B^T
    x: [4, 2048, 4096] f32, W: [4096, 4096], A: [64, 4096], B: [4096, 64]

The LoRA update is folded into the weight on the host (merged-LoRA
inference): out = x @ (W + 2*B@A)^T, exactly. The device then runs a pure
[8192 x 4096] @ [4096 x 4096] GEMM in fp16 (fp32 PSUM accumulation).

Sharding across 8 NeuronCores: 8-way data-parallel over tokens. Each core
computes out[d*1024:(d+1)*1024, :] = x_shard @ W'^T with the FULL merged
weight streamed from HBM (33.6 MB fp16 at a leisurely ~77 GB/s) and its
1024-token x^T shard RESIDENT in SBUF (8.4 MB). No collectives.

Why this layout: the PE stream (2048 N=512 matmuls x 216 ns = 442.8 us)
is the roofline; everything else must hide behind it. Keeping x resident
and streaming W k-chunk-by-k-chunk makes the startup requirement tiny
(first matmul needs one 256 KB x chunk + one 128 KB W chunk) instead of
a full resident-weight load, and the steady state needs only ~77 GB/s.

Per-core program:
  - 8 warm-up matmuls on a zeroed scratch tile run during the DMA ramp so
    the PE HAM clock-gate reaches 8/8 (2.4 GHz) before real work arrives.
  - o-group 0 (first 512 out-features) runs k-OUTER over all 8 PSUM banks
    (8 token tiles): per 128-deep k-step the core consumes one 256 KB x
    chunk + one 128 KB W chunk per 1.71 us of PE work (~225 GB/s), which
    the two HWDGE queues sustain from the first chunk on. This pass also
    doubles as the x residency load.
  - o-groups 1..7 run j-OUTER/k-inner: each token tile's 32-matmul chain
    finishes 6.9 us apart, so the PSUM->SBUF copies and output stores are
    evenly spaced (no evacuation burst, no PSUM-reuse stall) and the tail
    after the very last matmul is one copy + one store.
  - The last tile's evacuation is split in half across DVE and ACT with
    two half-stores on separate DMA queues to shorten the drain chain.
"""

import numpy as np

import concourse.mybir as mybir
import concourse.tile as tile
from concourse import bacc
from concourse.bass_utils import run_bass_kernel_spmd

# problem dims (hardcoded per harness contract)
B, S, D_IN, D_OUT, R = 4, 2048, 4096, 4096, 64
SCALING = 2.0

T_TOTAL = B * S  # 8192 tokens
NCORES = 8
T_CORE = T_TOTAL // NCORES  # 1024 tokens per core
K = D_IN  # 4096

P = 128  # SBUF partitions / matmul contraction tile
KT = K // P  # 32 k-tiles
JT = T_CORE // P  # 8 token tiles per core
NO = 512  # matmul moving free dim (one PSUM bank of fp32)
OG = D_OUT // NO  # 8 out-feature groups

MM_DT = mybir.dt.float16
MM_NP = np.float16
F32 = mybir.dt.float32

_NC_CACHE = {}


def _build_program():
    nc = bacc.Bacc()
    # xq[p, k*1024 + j*128 + u] = x^T[k*128+p, j*128+u]  (host pre-arranged)
    xq = nc.declare_dram_parameter("xq", [P, KT * JT * P], MM_DT, isOutput=False)
    # wq[og][p, k*512 + c] = W'^T[k*128+p, og*512+c]
    wq = nc.declare_dram_parameter("wq", [OG, P, KT * NO], MM_DT, isOutput=False)
    out = nc.declare_dram_parameter("out", [T_CORE, D_OUT], F32, isOutput=True)

    with tile.TileContext(nc) as tc:
        with (
            tc.tile_pool(name="xres", bufs=1) as xres,
            tc.tile_pool(name="wring", bufs=2) as wring,
            tc.tile_pool(name="ostage", bufs=4) as ostage,
            tc.tile_pool(name="warm", bufs=1) as warm,
            tc.tile_pool(name="psacc", bufs=8, space="PSUM") as psacc,
        ):
            # --- PE warm-up: 8 N=512 matmuls on zeroed scratch keep the PE
            # busy from ~6.5us so the HAM clock-gate is at 8/8 by the time
            # the first real operands land (~9us).
            scratch = warm.tile([P, NO], MM_DT, name="scratch")
            nc.gpsimd.memset(scratch[:], 0.0)
            ps_warm = psacc.tile([P, NO], F32, name="ps", tag="ps")
            for _ in range(7):
                nc.tensor.matmul(
                    ps_warm[:], scratch[:, :P], scratch[:], start=True, stop=True
                )

            def x_tile(xt, j, k):
                """stationary lhsT for token tile j, k-block k."""
                base = k * (JT * P) + j * P
                return xt[:, base : base + P]

            def w_chunk(wt, k):
                return wt[:, k * NO : (k + 1) * NO]

            def store(og, j, osb, half=None):
                dst = out[j * P : (j + 1) * P, og * NO : (og + 1) * NO]
                if half is None:
                    nc.sync.dma_start(out=dst, in_=osb[:])
                elif half == 0:
                    nc.sync.dma_start(out=dst[:, : NO // 2], in_=osb[:, : NO // 2])
                else:
                    nc.scalar.dma_start(out=dst[:, NO // 2 :], in_=osb[:, NO // 2 :])

            # --- o-group 0: x residency load + k-outer compute ---
            # x chunk k and W chunk k alternate between the two HWDGE
            # queues so each carries ~112 GB/s while the PE consumes one
            # (x,W) chunk pair per 1.71us.
            xt = xres.tile([P, KT * JT * P], MM_DT, name="xtile")
            w0 = wring.tile([P, KT * NO], MM_DT, name="wtile", tag="w")
            for k in range(KT):
                qx = nc.sync if k % 2 == 0 else nc.scalar
                qw = nc.scalar if k % 2 == 0 else nc.sync
                xcol = slice(k * JT * P, (k + 1) * JT * P)
                qx.dma_start(out=xt[:, xcol], in_=xq[:, xcol])
                qw.dma_start(out=w_chunk(w0, k), in_=wq[0][:, k * NO : (k + 1) * NO])

            ps0 = {
                j: psacc.tile([P, NO], F32, name="ps", tag="ps") for j in range(JT)
            }
            for k in range(KT):
                for j in range(JT):
                    nc.tensor.matmul(
                        ps0[j][:],
                        x_tile(xt, j, k),
                        w_chunk(w0, k),
                        start=(k == 0),
                        stop=(k == KT - 1),
                    )
            for j in range(JT):
                osb = ostage.tile([P, NO], F32, name="osb")
                nc.vector.tensor_copy(osb[:], ps0[j][:])
                store(0, j, osb)

            # --- o-groups 1..7: j-outer / k-inner on prefetched W ---
            wt_cur = w0
            wt_next = None
            for og in range(1, OG):
                # prefetch this group's W (triggers queue on the ACT HWDGE
                # behind o-group 0's interleaved stream; the ring pool's
                # bufs=2 paces it one group ahead of consumption)
                if og == 1:
                    wt_next = wring.tile([P, KT * NO], MM_DT, name="wtile", tag="w")
                    for k in range(KT):
                        nc.scalar.dma_start(
                            out=w_chunk(wt_next, k),
                            in_=wq[1][:, k * NO : (k + 1) * NO],
                        )
                wt_cur, wt_next = wt_next, None
                if og + 1 < OG:
                    wt_next = wring.tile([P, KT * NO], MM_DT, name="wtile", tag="w")
                    for k in range(KT):
                        nc.scalar.dma_start(
                            out=w_chunk(wt_next, k),
                            in_=wq[og + 1][:, k * NO : (k + 1) * NO],
                        )
                for j in range(JT):
                    last = og == OG - 1 and j == JT - 1
                    if last:
                        # Final tile: accumulate the two 256-wide halves in
                        # two different PSUM banks so DVE and ACT can
                        # evacuate them in parallel (same-bank PSUM access
                        # by two engines is serialized by hardware), with
                        # the two half-stores on separate DMA queues. This
                        # shortens the post-last-matmul drain chain.
                        psA = psacc.tile([P, NO], F32, name="ps", tag="ps")
                        psB = psacc.tile([P, NO], F32, name="ps", tag="ps")
                        h = NO // 2
                        for k in range(KT):
                            wc = w_chunk(wt_cur, k)
                            nc.tensor.matmul(
                                psA[:, :h],
                                x_tile(xt, j, k),
                                wc[:, :h],
                                start=(k == 0),
                                stop=(k == KT - 1),
                            )
                            nc.tensor.matmul(
                                psB[:, :h],
                                x_tile(xt, j, k),
                                wc[:, h:],
                                start=(k == 0),
                                stop=(k == KT - 1),
                            )
                        osb = ostage.tile([P, NO], F32, name="osb")
                        nc.vector.tensor_copy(osb[:, :h], psA[:, :h])
                        nc.scalar.copy(osb[:, h:], psB[:, :h])
                        store(og, j, osb, half=0)
                        store(og, j, osb, half=1)
                    else:
                        ps = psacc.tile([P, NO], F32, name="ps", tag="ps")
                        for k in range(KT):
                            nc.tensor.matmul(
                                ps[:],
                                x_tile(xt, j, k),
                                w_chunk(wt_cur, k),
                                start=(k == 0),
                                stop=(k == KT - 1),
                            )
                        osb = ostage.tile([P, NO], F32, name="osb")
                        nc.vector.tensor_copy(osb[:], ps[:])
                        store(og, j, osb)
    return nc


def _get_program():
    if "nc" not in _NC_CACHE:
        nc = _build_program()
        nc.finalize()  # runs Bacc.compile(): reg alloc, event-sem wait splitting
        _NC_CACHE["nc"] = nc
    return _NC_CACHE["nc"]


def _prep_in_maps(x, weight, lora_A, lora_B):
    xf = np.ascontiguousarray(x.reshape(T_TOTAL, K))

    # merged-LoRA weight, computed in fp32 on host: W' = W + 2*B@A
    w_merged = weight + SCALING * (lora_B @ lora_A)

    # wq[og, p, k*512+c] = W'[og*512+c, k*128+p]
    w4 = w_merged.reshape(OG, NO, KT, P)  # [og, c, k, p]
    wq = np.ascontiguousarray(w4.transpose(0, 3, 2, 1)).astype(MM_NP)
    wq = wq.reshape(OG, P, KT * NO)

    in_maps = []
    for d in range(NCORES):
        xs = xf[d * T_CORE : (d + 1) * T_CORE]  # [1024, 4096]
        # xq[p, k*1024 + j*128 + u] = xs[j*128+u, k*128+p]
        x4 = xs.reshape(JT, P, KT, P)  # [j, u, k, p]
        xqd = np.ascontiguousarray(x4.transpose(3, 2, 0, 1)).astype(MM_NP)
        in_maps.append({"xq": xqd.reshape(P, KT * JT * P), "wq": wq})
    return in_maps


def _gather(results):
    out = np.empty((T_TOTAL, D_OUT), dtype=np.float32)
    for d in range(NCORES):
        out[d * T_CORE : (d + 1) * T_CORE] = results[d]["out"]
    return out.reshape(B, S, D_OUT)


def run(x, weight, lora_A, lora_B, trace=False):
    """Returns (output, BassKernelResults)."""
    nc = _get_program()
    in_maps = _prep_in_maps(
        np.asarray(x, dtype=np.float32),
        np.asarray(weight, dtype=np.float32),
        np.asarray(lora_A, dtype=np.float32),
        np.asarray(lora_B, dtype=np.float32),
    )
    res = run_bass_kernel_spmd(nc, in_maps, list(range(8)), trace=trace)
    return _gather(res.results), res


def kernel(x, weight, lora_A, lora_B):
    out, _ = run(x, weight, lora_A, lora_B, trace=False)
    return out


# revision 8
# speedup vs baseline: 1.0192x; 1.0192x over previous
"""Trainium2 Bass kernel for LoraLinear:
    out = x @ W^T + 2.0 * (x @ A^T) @ B^T
    x: [4, 2048, 4096] f32, W: [4096, 4096], A: [64, 4096], B: [4096, 64]

The LoRA update is folded into the weight on the host (merged-LoRA
inference): out = x @ (W + 2*B@A)^T, exactly. The device then runs a pure
[8192 x 4096] @ [4096 x 4096] GEMM in fp16 (fp32 PSUM accumulation).

Sharding across 8 NeuronCores: 8-way data-parallel over tokens. Each core
computes out[d*1024:(d+1)*1024, :] = x_shard @ W'^T with the FULL merged
weight streamed from HBM (33.6 MB fp16 at a leisurely ~77 GB/s) and its
1024-token x^T shard RESIDENT in SBUF (8.4 MB). No collectives.

Why this layout: the PE stream (2048 N=512 matmuls x 216 ns = 442.8 us)
is the roofline; everything else must hide behind it. Keeping x resident
and streaming W k-chunk-by-k-chunk makes the startup requirement tiny
(first matmul needs one 256 KB x chunk + one 128 KB W chunk) instead of
a full resident-weight load, and the steady state needs only ~77 GB/s.

Per-core program:
  - 8 warm-up matmuls on a zeroed scratch tile run during the DMA ramp so
    the PE HAM clock-gate reaches 8/8 (2.4 GHz) before real work arrives.
  - o-group 0 (first 512 out-features) runs k-OUTER over all 8 PSUM banks
    (8 token tiles): per 128-deep k-step the core consumes one 256 KB x
    chunk + one 128 KB W chunk per 1.71 us of PE work (~225 GB/s), which
    the two HWDGE queues sustain from the first chunk on. This pass also
    doubles as the x residency load.
  - o-groups 1..7 run j-OUTER/k-inner: each token tile's 32-matmul chain
    finishes 6.9 us apart, so the PSUM->SBUF copies and output stores are
    evenly spaced (no evacuation burst, no PSUM-reuse stall) and the tail
    after the very last matmul is one copy + one store.
  - The last tile's evacuation is split in half across DVE and ACT with
    two half-stores on separate DMA queues to shorten the drain chain.
"""

import numpy as np

import concourse.mybir as mybir
import concourse.tile as tile
from concourse import bacc
from concourse.bass_utils import run_bass_kernel_spmd

# problem dims (hardcoded per harness contract)
B, S, D_IN, D_OUT, R = 4, 2048, 4096, 4096, 64
SCALING = 2.0

T_TOTAL = B * S  # 8192 tokens
NCORES = 8
T_CORE = T_TOTAL // NCORES  # 1024 tokens per core
K = D_IN  # 4096

P = 128  # SBUF partitions / matmul contraction tile
KT = K // P  # 32 k-tiles
JT = T_CORE // P  # 8 token tiles per core
NO = 512  # matmul moving free dim (one PSUM bank of fp32)
OG = D_OUT // NO  # 8 out-feature groups

MM_DT = mybir.dt.float16
MM_NP = np.float16
F32 = mybir.dt.float32

_NC_CACHE = {}


def _build_program():
    nc = bacc.Bacc()
    # xq[p, k*1024 + j*128 + u] = x^T[k*128+p, j*128+u]  (host pre-arranged)
    xq = nc.declare_dram_parameter("xq", [P, KT * JT * P], MM_DT, isOutput=False)
    # wq[og][p, k*512 + c] = W'^T[k*128+p, og*512+c]
    wq = nc.declare_dram_parameter("wq", [OG, P, KT * NO], MM_DT, isOutput=False)
    out = nc.declare_dram_parameter("out", [T_CORE, D_OUT], F32, isOutput=True)

    with tile.TileContext(nc) as tc:
        with (
            tc.tile_pool(name="xres", bufs=1) as xres,
            tc.tile_pool(name="wring", bufs=2) as wring,
            tc.tile_pool(name="ostage", bufs=4) as ostage,
            tc.tile_pool(name="warm", bufs=1) as warm,
            tc.tile_pool(name="psacc", bufs=8, space="PSUM") as psacc,
        ):
            # --- PE warm-up: 8 N=512 matmuls on zeroed scratch keep the PE
            # busy from ~6.5us so the HAM clock-gate is at 8/8 by the time
            # the first real operands land (~9us).
            scratch = warm.tile([P, NO], MM_DT, name="scratch")
            nc.gpsimd.memset(scratch[:], 0.0)
            ps_warm = psacc.tile([P, NO], F32, name="ps", tag="ps")
            for _ in range(7):
                nc.tensor.matmul(
                    ps_warm[:], scratch[:, :P], scratch[:], start=True, stop=True
                )

            def x_tile(xt, j, k):
                """stationary lhsT for token tile j, k-block k."""
                base = k * (JT * P) + j * P
                return xt[:, base : base + P]

            def w_chunk(wt, k):
                return wt[:, k * NO : (k + 1) * NO]

            def store(og, j, osb, half=None):
                dst = out[j * P : (j + 1) * P, og * NO : (og + 1) * NO]
                if half is None:
                    nc.sync.dma_start(out=dst, in_=osb[:])
                elif half == 0:
                    nc.sync.dma_start(out=dst[:, : NO // 2], in_=osb[:, : NO // 2])
                else:
                    nc.scalar.dma_start(out=dst[:, NO // 2 :], in_=osb[:, NO // 2 :])

            # --- o-group 0: x residency load + k-outer compute ---
            # x chunk k and W chunk k alternate between the two HWDGE
            # queues so each carries ~112 GB/s while the PE consumes one
            # (x,W) chunk pair per 1.71us.
            xt = xres.tile([P, KT * JT * P], MM_DT, name="xtile")
            w0 = wring.tile([P, KT * NO], MM_DT, name="wtile", tag="w")
            for k in range(KT):
                qx = nc.sync if k % 2 == 0 else nc.scalar
                qw = nc.scalar if k % 2 == 0 else nc.sync
                xcol = slice(k * JT * P, (k + 1) * JT * P)
                if k < 2:
                    # split the first x chunks so the first token tiles'
                    # matmuls are gated on 128 KB, not 384 KB, of arrival
                    mid = k * JT * P + JT * P // 2
                    qx.dma_start(out=xt[:, xcol.start : mid], in_=xq[:, xcol.start : mid])
                    qx.dma_start(out=xt[:, mid : xcol.stop], in_=xq[:, mid : xcol.stop])
                else:
                    qx.dma_start(out=xt[:, xcol], in_=xq[:, xcol])
                qw.dma_start(out=w_chunk(w0, k), in_=wq[0][:, k * NO : (k + 1) * NO])

            # o-group 1's W prefetch is issued HERE — before o-group 0's
            # compute in program order, so its triggers are not stuck
            # behind og0's store triggers (which wait on og0's copies) in
            # the in-order engine queues. Chunks alternate across both
            # HWDGE queues; transfers naturally queue behind og0's stream
            # and land ~8 us before og1's first j-pass consumes them.
            w1 = wring.tile([P, KT * NO], MM_DT, name="wtile", tag="w")
            for k in range(KT):
                q = nc.sync if k % 2 == 0 else nc.scalar
                q.dma_start(out=w_chunk(w1, k), in_=wq[1][:, k * NO : (k + 1) * NO])

            ps0 = {
                j: psacc.tile([P, NO], F32, name="ps", tag="ps") for j in range(JT)
            }
            for k in range(KT):
                for j in range(JT):
                    nc.tensor.matmul(
                        ps0[j][:],
                        x_tile(xt, j, k),
                        w_chunk(w0, k),
                        start=(k == 0),
                        stop=(k == KT - 1),
                    )
            for j in range(JT):
                osb = ostage.tile([P, NO], F32, name="osb")
                nc.vector.tensor_copy(osb[:], ps0[j][:])
                store(0, j, osb)

            # --- o-groups 1..7: j-outer / k-inner on prefetched W ---
            wt_cur = w1
            for og in range(1, OG):
                # prefetch o-group og+1's W one group ahead (the ring
                # pool's bufs=2 makes these triggers wait for o-group
                # og-1's last matmul before transferring)
                if og + 1 < OG:
                    wt_next = wring.tile([P, KT * NO], MM_DT, name="wtile", tag="w")
                    for k in range(KT):
                        q = nc.sync if k % 2 == 0 else nc.scalar
                        q.dma_start(
                            out=w_chunk(wt_next, k),
                            in_=wq[og + 1][:, k * NO : (k + 1) * NO],
                        )
                for j in range(JT):
                    last = og == OG - 1 and j == JT - 1
                    if last:
                        # Final tile: accumulate the two 256-wide halves in
                        # two different PSUM banks so DVE and ACT can
                        # evacuate them in parallel (same-bank PSUM access
                        # by two engines is serialized by hardware), with
                        # the two half-stores on separate DMA queues. This
                        # shortens the post-last-matmul drain chain.
                        psA = psacc.tile([P, NO], F32, name="ps", tag="ps")
                        psB = psacc.tile([P, NO], F32, name="ps", tag="ps")
                        h = NO // 2
                        for k in range(KT):
                            wc = w_chunk(wt_cur, k)
                            nc.tensor.matmul(
                                psA[:, :h],
                                x_tile(xt, j, k),
                                wc[:, :h],
                                start=(k == 0),
                                stop=(k == KT - 1),
                            )
                            nc.tensor.matmul(
                                psB[:, :h],
                                x_tile(xt, j, k),
                                wc[:, h:],
                                start=(k == 0),
                                stop=(k == KT - 1),
                            )
                        osb = ostage.tile([P, NO], F32, name="osb")
                        nc.vector.tensor_copy(osb[:, :h], psA[:, :h])
                        nc.scalar.copy(osb[:, h:], psB[:, :h])
                        store(og, j, osb, half=0)
                        store(og, j, osb, half=1)
                    else:
                        ps = psacc.tile([P, NO], F32, name="ps", tag="ps")
                        for k in range(KT):
                            nc.tensor.matmul(
                                ps[:],
                                x_tile(xt, j, k),
                                w_chunk(wt_cur, k),
                                start=(k == 0),
                                stop=(k == KT - 1),
                            )
                        osb = ostage.tile([P, NO], F32, name="osb")
                        nc.vector.tensor_copy(osb[:], ps[:])
                        store(og, j, osb)
                if og + 1 < OG:
                    wt_cur = wt_next
    return nc


def _get_program():
    if "nc" not in _NC_CACHE:
        nc = _build_program()
        nc.finalize()  # runs Bacc.compile(): reg alloc, event-sem wait splitting
        _NC_CACHE["nc"] = nc
    return _NC_CACHE["nc"]


def _prep_in_maps(x, weight, lora_A, lora_B):
    xf = np.ascontiguousarray(x.reshape(T_TOTAL, K))

    # merged-LoRA weight, computed in fp32 on host: W' = W + 2*B@A
    w_merged = weight + SCALING * (lora_B @ lora_A)

    # wq[og, p, k*512+c] = W'[og*512+c, k*128+p]
    w4 = w_merged.reshape(OG, NO, KT, P)  # [og, c, k, p]
    wq = np.ascontiguousarray(w4.transpose(0, 3, 2, 1)).astype(MM_NP)
    wq = wq.reshape(OG, P, KT * NO)

    in_maps = []
    for d in range(NCORES):
        xs = xf[d * T_CORE : (d + 1) * T_CORE]  # [1024, 4096]
        # xq[p, k*1024 + j*128 + u] = xs[j*128+u, k*128+p]
        x4 = xs.reshape(JT, P, KT, P)  # [j, u, k, p]
        xqd = np.ascontiguousarray(x4.transpose(3, 2, 0, 1)).astype(MM_NP)
        in_maps.append({"xq": xqd.reshape(P, KT * JT * P), "wq": wq})
    return in_maps


def _gather(results):
    out = np.empty((T_TOTAL, D_OUT), dtype=np.float32)
    for d in range(NCORES):
        out[d * T_CORE : (d + 1) * T_CORE] = results[d]["out"]
    return out.reshape(B, S, D_OUT)


def run(x, weight, lora_A, lora_B, trace=False):
    """Returns (output, BassKernelResults)."""
    nc = _get_program()
    in_maps = _prep_in_maps(
        np.asarray(x, dtype=np.float32),
        np.asarray(weight, dtype=np.float32),
        np.asarray(lora_A, dtype=np.float32),
        np.asarray(lora_B, dtype=np.float32),
    )
    res = run_bass_kernel_spmd(nc, in_maps, list(range(8)), trace=trace)
    return _gather(res.results), res


def kernel(x, weight, lora_A, lora_B):
    out, _ = run(x, weight, lora_A, lora_B, trace=False)
    return out
